# revision 1
# baseline (speedup 1.0000x reference)
"""DFlashDraftModel Trainium2 kernel — 8 NeuronCores, DP2 x TP4.

Sharding: cores 0-3 handle batches 0-1, cores 4-7 handle batches 2-3 (DP groups).
Within each group of 4 (TP): fc output-dim sharded (640/core) + AllGather;
q heads 5/core, kv heads 1/core, o input-sharded, gate/up 1728/core,
down input-sharded; AllReduce after o_proj and down_proj within the group.

All matmuls bf16 with fp32 PSUM accumulation. RMS-norm scales are commuted
through matmuls (applied to outputs); norm weight vectors are folded into
weights / rope tables / host post-scale.

v2: one-pass KV build for all 5 layers (th read once, K/V SBUF-resident),
decoder weights loaded via few large block DMAs, bf16 AllReduces.
"""
import sys

if '/opt/trn_rl_repo' not in sys.path:
    sys.path.insert(0, '/opt/trn_rl_repo')

import numpy as np
import ml_dtypes

import concourse.bass as bass
import concourse.tile as tile
from concourse import bacc, mybir
from concourse import bass_utils
from concourse.masks import make_identity

BF16 = mybir.dt.bfloat16
F32 = mybir.dt.float32
AF = mybir.ActivationFunctionType
OP = mybir.AluOpType

L = 5; B = 4; Q = 16; CTX = 2048; S = CTX + Q
H = 2560; I = 6912; NH = 20; NKV = 4; HD = 128; NF = 5
EPS = 1e-6; THETA = 1000000.0

NCORES = 8
TP = 4                      # tensor-parallel group size
GROUPS = [[0, 1, 2, 3], [4, 5, 6, 7]]
TOK = 2 * CTX               # ctx tokens per core (2 batches)
NTC = TOK // 128            # 32 context token chunks per core
FIN = NF * H                # 12800
KF = FIN // 128             # 100 fc contraction chunks
HSH = H // TP               # 640  fc output shard per core
ISH = I // TP               # 1728 gate/up shard
QH = NH // TP               # 5 q heads per core
DTOK = 2 * Q                # 32 decoder tokens per core
NKC = 20                    # H/128 contraction chunks
NFC = 8                     # fc token chunks (512 each)
FCN = 512                   # fc token chunk width
GU2 = 2 * ISH               # 3456
# gate/up interleaved column chunks: [g0,u0,g1,u1,g2,u2,(g3|u3)]
GU_CH = [512, 512, 512, 512, 512, 512, 384]
DKC = 14                    # down contraction chunks: 13*128 + 64
DK_LAST = ISH - 13 * 128    # 64
DWP = DKC * 128             # padded down contraction rows (1792)


def _build():
    nc = bacc.Bacc("TRN2", target_bir_lowering=False, debug=False,
                   enable_asserts=False, num_devices=NCORES)

    # ---- I/O ----
    xt_d = nc.dram_tensor("xt", [FIN, TOK], BF16, kind="ExternalInput").ap()
    fcw_d = nc.dram_tensor("fcw", [FIN, HSH], BF16, kind="ExternalInput").ap()
    kvw_d = nc.dram_tensor("kvw", [L, H, 256], BF16, kind="ExternalInput").ap()
    qkvw_d = nc.dram_tensor("qkvw", [L, H, 896], BF16, kind="ExternalInput").ap()
    ow_d = nc.dram_tensor("ow", [L, HSH, H], BF16, kind="ExternalInput").ap()
    guw_d = nc.dram_tensor("guw", [L, H, GU2], BF16, kind="ExternalInput").ap()
    dw_d = nc.dram_tensor("dw", [L, DWP, H], BF16, kind="ExternalInput").ap()
    ckc_d = nc.dram_tensor("ckc", [L, 2, CTX, HD], BF16, kind="ExternalInput").ap()
    skc_d = nc.dram_tensor("skc", [L, 2, CTX, HD], BF16, kind="ExternalInput").ap()
    ckq_d = nc.dram_tensor("ckq", [L, DTOK, HD], BF16, kind="ExternalInput").ap()
    skq_d = nc.dram_tensor("skq", [L, DTOK, HD], BF16, kind="ExternalInput").ap()
    ckd_d = nc.dram_tensor("ckd", [L, DTOK, HD], BF16, kind="ExternalInput").ap()
    skd_d = nc.dram_tensor("skd", [L, DTOK, HD], BF16, kind="ExternalInput").ap()
    x0_d = nc.dram_tensor("x0", [DTOK, H], F32, kind="ExternalInput").ap()
    y_d = nc.dram_tensor("y", [DTOK, H], F32, kind="ExternalOutput").ap()

    with tile.TileContext(nc) as tc:
        _emit(nc, tc, xt_d, fcw_d, kvw_d, qkvw_d, ow_d, guw_d, dw_d,
              ckc_d, skc_d, ckq_d, skq_d, ckd_d, skd_d, x0_d, y_d)
    nc.compile()
    return nc


def _rsqrt(nc, pool, ss, n_mean, ptok, name):
    """rsqrt(ss/n_mean + EPS) for per-partition scalars ss [ptok,1] f32."""
    m = pool.tile([ptok, 1], F32, name=f"m_{name}")
    nc.vector.tensor_scalar(out=m[:], in0=ss[:], scalar1=1.0 / n_mean,
                            scalar2=EPS, op0=OP.mult, op1=OP.add)
    r = pool.tile([ptok, 1], F32, name=f"r_{name}")
    nc.vector.reciprocal(r[:], m[:])
    o = pool.tile([ptok, 1], F32, name=f"o_{name}")
    nc.scalar.sqrt(o[:], r[:])
    return o


def _rope(nc, pool, src, ck, sk, ptok, name):
    """out = src*ck + rotate_half_pair(src)*sk  (sign folded into sk).
    src/ck/sk: [ptok, 128] bf16 APs. Returns bf16 tile."""
    t1 = pool.tile([ptok, HD], BF16, name=f"t1_{name}")
    nc.vector.tensor_tensor(out=t1[:], in0=src, in1=ck, op=OP.mult)
    t2 = pool.tile([ptok, HD], BF16, name=f"t2_{name}")
    nc.vector.tensor_tensor(out=t2[:, 0:64], in0=src[:, 64:128],
                            in1=sk[:, 0:64], op=OP.mult)
    nc.vector.tensor_tensor(out=t2[:, 64:128], in0=src[:, 0:64],
                            in1=sk[:, 64:128], op=OP.mult)
    t3 = pool.tile([ptok, HD], BF16, name=f"t3_{name}")
    nc.vector.tensor_tensor(out=t3[:], in0=t1[:], in1=t2[:], op=OP.add)
    return t3


def _emit(nc, tc, xt_d, fcw_d, kvw_d, qkvw_d, ow_d, guw_d, dw_d,
          ckc_d, skc_d, ckq_d, skq_d, ckd_d, skd_d, x0_d, y_d):
    with tc.tile_pool(name="glob", bufs=1) as glob, \
         tc.tile_pool(name="dramg", bufs=1, space="DRAM") as dram:
        # ---------------- long-lived tiles ----------------
        ident = glob.tile([128, 128], BF16, name="ident")
        make_identity(nc, ident[:])
        # shifted identity: ident_sh[16+j, j] = 1 (selects rows 16:32)
        ident_sh = glob.tile([128, 128], BF16, name="ident_sh")
        nc.gpsimd.memset(ident_sh[:], 0.0)
        nc.gpsimd.affine_select(out=ident_sh[:], in_=ident_sh[:],
                                compare_op=OP.not_equal, fill=1.0, base=-16,
                                pattern=[[-1, 128]], channel_multiplier=1)
        ones = glob.tile([128, 1], BF16, name="ones")
        nc.vector.memset(ones[:], 1.0)
        i1f = glob.tile([1, 1], F32, name="i1f")
        nc.vector.memset(i1f[:], 1.0)
        x_sb = glob.tile([DTOK, H], F32, name="x_sb")
        nc.sync.dma_start(out=x_sb[:], in_=x0_d[:])
        invT = glob.tile([128, NTC], F32, name="invT")

        th_n = [dram.tile([H, FCN], BF16, name=f"th{n}") for n in range(NFC)]
        ssq_in = dram.tile([1, TOK], F32, name="ssq_in")
        ssq_out = dram.tile([1, TOK], F32, name="ssq_out")

        # ---------------- Phase 1: fc ----------------
        with tc.tile_pool(name="fcw_pool", bufs=1) as fcwp, \
             tc.tile_pool(name="xtp", bufs=6) as xtp, \
             tc.tile_pool(name="fcps", bufs=7, space="PSUM") as fcps, \
             tc.tile_pool(name="ssqps", bufs=1, space="PSUM") as ssqps, \
             tc.tile_pool(name="fcev", bufs=4) as fcev, \
             tc.tile_pool(name="ssqrp", bufs=1) as ssqrp, \
             tc.tile_pool(name="agp", bufs=2, space="DRAM") as agp:
            fcw_sb = fcwp.tile([128, KF, HSH], BF16)
            ssq_row = ssqrp.tile([1, TOK], F32, name="ssq_row")
            fcw_r = fcw_d.rearrange("(kk p) m -> p kk m", p=128)
            for qq in range(4):
                nc.sync.dma_start(
                    out=fcw_sb[:, 25 * qq:25 * (qq + 1), :],
                    in_=fcw_r[:, 25 * qq:25 * (qq + 1), :])
            for n in range(NFC):
                psF = [fcps.tile([128, FCN], F32, name="psF", tag="psF")
                       for _ in range(5)]
                for k in range(KF):
                    xt_t = xtp.tile([128, FCN], BF16, name="xt_t", tag="xt_t")
                    nc.sync.dma_start(
                        out=xt_t[:],
                        in_=xt_d[128 * k:128 * (k + 1), FCN * n:FCN * (n + 1)])
                    for m in range(5):
                        nc.tensor.matmul(
                            psF[m][:], fcw_sb[:, k, 128 * m:128 * (m + 1)],
                            xt_t[:], start=(k == 0), stop=(k == KF - 1))
                ag_in = agp.tile([HSH, FCN], BF16, name="ag_in", tag="ag_in")
                ssq_ps = ssqps.tile([1, FCN], F32, name="ssq_ps", tag="ssq_ps")
                for m in range(5):
                    th_bf = fcev.tile([128, FCN], BF16, name="th_bf", tag="th_bf")
                    nc.vector.tensor_copy(th_bf[:], psF[m][:])
                    nc.sync.dma_start(out=ag_in[128 * m:128 * (m + 1), :],
                                      in_=th_bf[:])
                    th2 = fcev.tile([128, FCN], BF16, name="th2", tag="th2")
                    nc.scalar.square(th2[:], psF[m][:])
                    nc.tensor.matmul(ssq_ps[:], ones[:], th2[:],
                                     start=(m == 0), stop=(m == 4))
                nc.vector.tensor_copy(ssq_row[:, FCN * n:FCN * (n + 1)], ssq_ps[:])
                nc.gpsimd.collective_compute(
                    "AllGather", OP.bypass, replica_groups=GROUPS,
                    ins=[ag_in[:]], outs=[th_n[n][:]])
            # sum-of-squares all-reduce + inv_rms in [128, NTC] layout
            nc.sync.dma_start(out=ssq_in[:], in_=ssq_row[:])
            nc.gpsimd.collective_compute(
                "AllReduce", OP.add, replica_groups=GROUPS,
                ins=[ssq_in[:]], outs=[ssq_out[:]])

        with tc.tile_pool(name="kvglob", bufs=1) as kvglob:
            # K/V for all layers, SBUF-resident (allocated after fc frees SBUF)
            KT_all = kvglob.tile([128, L, 2, S], BF16, name="KT_all")
            V_all = kvglob.tile([128, L, 2, 17, HD], BF16, name="V_all")
            # ---------------- Phase 2: one-pass KV build, all layers ----------------
            with tc.tile_pool(name="kvwp", bufs=1) as kvwp, \
                 tc.tile_pool(name="tht", bufs=3) as thtp, \
                 tc.tile_pool(name="ropep", bufs=3) as ropep, \
                 tc.tile_pool(name="kvps", bufs=5, space="PSUM") as kvps, \
                 tc.tile_pool(name="ktps", bufs=2, space="PSUM") as ktps, \
                 tc.tile_pool(name="kvpost", bufs=4) as kvpost:
                kvw_sb = kvwp.tile([128, NKC, L, 256], BF16, name="kvw_sb")
                for l in range(L):
                    nc.sync.dma_start(
                        out=kvw_sb[:, :, l, :],
                        in_=kvw_d[l].rearrange("(kk p) c -> p kk c", p=128))
                ckc_r = ckc_d.rearrange("l b (c p) d -> p l (b c) d", p=128)
                skc_r = skc_d.rearrange("l b (c p) d -> p l (b c) d", p=128)
                for t_ in range(NTC):
                    if t_ == 10:
                        # inv_rms of th — emitted mid-pass so the engine queues
                        # never stall on the ssq AllReduce result
                        with tc.tile_pool(name="ivt2", bufs=1) as ivt2:
                            ssT = ivt2.tile([128, NTC], F32, name="ssT")
                            nc.sync.dma_start(
                                out=ssT[:],
                                in_=ssq_out.rearrange("o (c p) -> (o p) c",
                                                      p=128))
                            m = ivt2.tile([128, NTC], F32, name="m_iv")
                            nc.vector.tensor_scalar(out=m[:], in0=ssT[:],
                                                    scalar1=1.0 / H,
                                                    scalar2=EPS,
                                                    op0=OP.mult, op1=OP.add)
                            r = ivt2.tile([128, NTC], F32, name="r_iv")
                            nc.vector.reciprocal(r[:], m[:])
                            nc.scalar.sqrt(invT[:], r[:])
                    b, c = t_ // 16, t_ % 16
                    thT_t = thtp.tile([128, NKC, 128], BF16, name="thT_t",
                                      tag="thT")
                    nc.sync.dma_start(
                        out=thT_t[:],
                        in_=th_n[t_ // 4].rearrange(
                            "(kk p) (cc w) -> p kk cc w", p=128, w=128)
                            [:, :, t_ % 4, :])
                    ck_t = ropep.tile([128, L, HD], BF16, name="ck_t", tag="ck_t")
                    nc.sync.dma_start(out=ck_t[:], in_=ckc_r[:, :, t_, :])
                    sk_t = ropep.tile([128, L, HD], BF16, name="sk_t", tag="sk_t")
                    nc.sync.dma_start(out=sk_t[:], in_=skc_r[:, :, t_, :])
                    kv_ps = [kvps.tile([128, 256], F32, name="kv_ps", tag="kv_ps")
                             for _ in range(L)]
                    if t_ == 0:
                        # layer-major on the first chunk: start matmuls as soon
                        # as layer-0 weights land (hides the kvw load latency)
                        for l in range(L):
                            for k in range(NKC):
                                nc.tensor.matmul(kv_ps[l][:], thT_t[:, k, :],
                                                 kvw_sb[:, k, l, :],
                                                 start=(k == 0),
                                                 stop=(k == NKC - 1))
                    else:
                        for k in range(NKC):
                            for l in range(L):
                                nc.tensor.matmul(kv_ps[l][:], thT_t[:, k, :],
                                                 kvw_sb[:, k, l, :],
                                                 start=(k == 0),
                                                 stop=(k == NKC - 1))
                    for l in range(L):
                        kvs = kvpost.tile([128, 256], BF16, name="kvs", tag="kvs")
                        nc.vector.tensor_copy(kvs[:], kv_ps[l][:])
                        nc.vector.tensor_copy(V_all[:, l, b, c, :], kvs[:, 128:256])
                        k2t = kvpost.tile([128, HD], BF16, name="k2t", tag="k2t")
                        ks = kvpost.tile([128, 1], F32, name="ks", tag="ks")
                        nc.scalar.activation(k2t[:], kvs[:, 0:128], AF.Square,
                                             accum_out=ks[:])
                        rs = _rsqrt(nc, kvpost, ks, HD, 128, "kh")
                        khn = kvpost.tile([128, HD], BF16, name="khn", tag="khn")
                        nc.vector.tensor_scalar_mul(khn[:], kvs[:, 0:128], rs[:])
                        kr = _rope(nc, kvpost, khn[:], ck_t[:, l, :],
                                   sk_t[:, l, :], 128, "kc")
                        kt_ps = ktps.tile([128, 128], BF16, name="kt_ps",
                                          tag="kt_ps")
                        nc.tensor.transpose(kt_ps[:], kr[:], ident[:])
                        nc.vector.tensor_copy(
                            KT_all[:, l, b, 128 * c:128 * (c + 1)], kt_ps[:])
                # deferred inv_rms(th) scaling of ctx V (kept out of the hot
                # KV loop so it does not serialize on the ssq AllReduce)
                for t_ in range(NTC):
                    b, c = t_ // 16, t_ % 16
                    nc.vector.tensor_scalar_mul(
                        V_all[:, :, b, c, :], V_all[:, :, b, c, :],
                        invT[:, t_:t_ + 1])

            # ---------------- Phase 3: decoder layers ----------------
            with tc.tile_pool(name="sc1", bufs=1) as sc1, \
                 tc.tile_pool(name="arb", bufs=1) as arb, \
                 tc.tile_pool(name="wblk", bufs=3) as wblkp, \
                 tc.tile_pool(name="dps", bufs=7, space="PSUM") as dps, \
                 tc.tile_pool(name="ktp2", bufs=1, space="PSUM") as ktp2, \
                 tc.tile_pool(name="attp", bufs=2) as attp, \
                 tc.tile_pool(name="decx", bufs=2) as decx, \
                 tc.tile_pool(name="ardr", bufs=2, space="DRAM") as ardr:
                qkvw_r = qkvw_d.rearrange("l (kk p) c -> l p kk c", p=128)
                ow_r = ow_d.rearrange("l (h p) m -> l p h m", p=128)
                guw_r = guw_d.rearrange("l (kk p) m -> l p kk m", p=128)
                dw_r = dw_d.rearrange("l (kk p) m -> l p kk m", p=128)
                for l in range(L):
                    # inv_rms1 of x
                    xsq = sc1.tile([DTOK, H], BF16, name="xsq", tag="xb")
                    ssx = sc1.tile([DTOK, 1], F32, name="ssx", tag="ssx")
                    nc.scalar.activation(xsq[:], x_sb[:], AF.Square, accum_out=ssx[:])
                    inv1 = _rsqrt(nc, sc1, ssx, H, DTOK, "in1")
                    # xT (raw x, bf16)
                    xb = sc1.tile([DTOK, H], BF16, name="xb", tag="xb")
                    nc.vector.tensor_copy(xb[:], x_sb[:])
                    xT = decx.tile([128, NKC, DTOK], BF16, name="xT", tag="xT")
                    for k in range(NKC):
                        tp = ktp2.tile([128, DTOK], BF16, name="tp_x", tag="kt_ps")
                        nc.tensor.transpose(tp[:], xb[:, 128 * k:128 * (k + 1)],
                                            ident[:DTOK, :DTOK])
                        nc.vector.tensor_copy(xT[:, k, :], tp[:])
                    # qkv projection — two half-block DMAs
                    q_ps = dps.tile([DTOK, 512], F32, name="q_ps", tag="acc")
                    q_ps2 = dps.tile([DTOK, 384], F32, name="q_ps2", tag="acc")
                    for hb in range(2):
                        qkv_w = wblkp.tile([128, 10, 896], BF16, name="qkv_w",
                                           tag="wblk")
                        nc.sync.dma_start(out=qkv_w[:],
                                          in_=qkvw_r[l, :, 10 * hb:10 * (hb + 1), :])
                        for k10 in range(10):
                            k = 10 * hb + k10
                            nc.tensor.matmul(q_ps[:], xT[:, k, :],
                                             qkv_w[:, k10, 0:512],
                                             start=(k == 0), stop=(k == NKC - 1))
                            nc.tensor.matmul(q_ps2[:], xT[:, k, :],
                                             qkv_w[:, k10, 512:896],
                                             start=(k == 0), stop=(k == NKC - 1))
                    # q: scale by inv1, per-head rms, rope
                    q1 = sc1.tile([DTOK, QH * HD], BF16, name="q1", tag="q1")
                    nc.vector.tensor_scalar_mul(q1[:, 0:512], q_ps[:], inv1[:])
                    nc.vector.tensor_scalar_mul(q1[:, 512:640], q_ps2[:, 0:128],
                                                inv1[:])
                    ckq_sb = sc1.tile([DTOK, HD], BF16, name="ckq_sb", tag="ckq")
                    skq_sb = sc1.tile([DTOK, HD], BF16, name="skq_sb", tag="skq")
                    nc.sync.dma_start(out=ckq_sb[:], in_=ckq_d[l])
                    nc.sync.dma_start(out=skq_sb[:], in_=skq_d[l])
                    qr = sc1.tile([DTOK, QH * HD], BF16, name="qr", tag="qr")
                    for h in range(QH):
                        hs = slice(128 * h, 128 * (h + 1))
                        q2h = sc1.tile([DTOK, HD], BF16, name="q2h", tag="q2h")
                        qsh = sc1.tile([DTOK, 1], F32, name="qsh", tag="qsh")
                        nc.scalar.activation(q2h[:], q1[:, hs], AF.Square,
                                             accum_out=qsh[:])
                        rqh = _rsqrt(nc, sc1, qsh, HD, DTOK, f"qh{h}")
                        qhn = sc1.tile([DTOK, HD], BF16, name="qhn", tag="qhn")
                        nc.vector.tensor_scalar_mul(qhn[:], q1[:, hs], rqh[:])
                        qrh = _rope(nc, sc1, qhn[:], ckq_sb[:], skq_sb[:], DTOK, "q")
                        nc.vector.tensor_copy(qr[:, hs], qrh[:])
                    # qT per batch
                    qT = [sc1.tile([128, QH * Q], BF16, name=f"qT{b}", tag=f"qT{b}")
                          for b in range(2)]
                    for b in range(2):
                        sel = ident[:DTOK, 0:Q] if b == 0 else ident_sh[:DTOK, 0:Q]
                        for h in range(QH):
                            tqp = ktp2.tile([128, Q], F32, name="tqp", tag="kt_ps")
                            nc.tensor.matmul(tqp[:], qr[:, 128 * h:128 * (h + 1)],
                                             sel, start=True, stop=True)
                            nc.vector.tensor_copy(qT[b][:, Q * h:Q * (h + 1)], tqp[:])
                    # kd/vd
                    kvd = sc1.tile([DTOK, 256], BF16, name="kvd", tag="kvd")
                    nc.vector.tensor_scalar_mul(kvd[:], q_ps2[:, 128:384], inv1[:])
                    k2d = sc1.tile([DTOK, HD], BF16, name="k2d", tag="k2d")
                    ksd = sc1.tile([DTOK, 1], F32, name="ksd", tag="ksd")
                    nc.scalar.activation(k2d[:], kvd[:, 0:128], AF.Square,
                                         accum_out=ksd[:])
                    rsd = _rsqrt(nc, sc1, ksd, HD, DTOK, "kd")
                    khd = sc1.tile([DTOK, HD], BF16, name="khd", tag="khd")
                    nc.vector.tensor_scalar_mul(khd[:], kvd[:, 0:128], rsd[:])
                    ckd_sb = sc1.tile([DTOK, HD], BF16, name="ckd_sb", tag="ckd")
                    skd_sb = sc1.tile([DTOK, HD], BF16, name="skd_sb", tag="skd")
                    nc.sync.dma_start(out=ckd_sb[:], in_=ckd_d[l])
                    nc.sync.dma_start(out=skd_sb[:], in_=skd_d[l])
                    krd = _rope(nc, sc1, khd[:], ckd_sb[:], skd_sb[:], DTOK, "kd")
                    for b in range(2):
                        sel = ident[:DTOK, 0:Q] if b == 0 else ident_sh[:DTOK, 0:Q]
                        kdp = ktp2.tile([128, Q], F32, name="kdp", tag="kt_ps")
                        nc.tensor.matmul(kdp[:], krd[:], sel, start=True, stop=True)
                        nc.vector.tensor_copy(KT_all[:, l, b, CTX:S], kdp[:])
                        # vd: partition shift via sbuf->sbuf dma
                        nc.sync.dma_start(out=V_all[0:Q, l, b, 16, :],
                                          in_=kvd[Q * b:Q * (b + 1), 128:256])
                    # prefetch o weights before attention (2 blocks)
                    ow_blks = []
                    for ob in range(2):
                        o0, oln = 3 * ob, (3 if ob == 0 else 2)
                        owt = wblkp.tile([128, 3, H], BF16, name="ow_sb",
                                         tag="wblk")
                        nc.sync.dma_start(out=owt[:, :oln, :],
                                          in_=ow_r[l, :, o0:o0 + oln, :])
                        ow_blks.append(owt)
                    # attention per batch
                    attn_cat = sc1.tile([128, 2 * QH * Q], BF16, name="attn_cat",
                                        tag="attn_cat")
                    for b in range(2):
                        rs_ps = dps.tile([1, QH * Q], F32, name="rs_ps", tag="acc")
                        ao_ps = dps.tile([QH * Q, HD], F32, name="ao_ps", tag="acc")
                        for s_ in range(17):
                            klen = 128 if s_ < 16 else Q
                            sc_ps = dps.tile([128, QH * Q], F32, name="sc_ps",
                                             tag="acc")
                            nc.tensor.matmul(
                                sc_ps[:klen, :],
                                KT_all[:, l, b, 128 * s_:128 * s_ + klen], qT[b][:],
                                start=True, stop=True)
                            at = attp.tile([128, QH * Q], BF16, name="at", tag="at")
                            nc.scalar.activation(at[:klen, :], sc_ps[:klen, :], AF.Exp)
                            nc.tensor.matmul(rs_ps[:], ones[:klen, :], at[:klen, :],
                                             start=(s_ == 0), stop=(s_ == 16))
                            nc.tensor.matmul(ao_ps[:], at[:klen, :],
                                             V_all[:klen, l, b, s_, :],
                                             start=(s_ == 0), stop=(s_ == 16))
                        rs_sb = sc1.tile([1, QH * Q], F32, name="rs_sb", tag="rs_sb")
                        nc.vector.tensor_copy(rs_sb[:], rs_ps[:])
                        rsT_ps = ktp2.tile([QH * Q, 1], F32, name="rsT_ps",
                                           tag="kt_ps")
                        nc.tensor.matmul(rsT_ps[:], rs_sb[:], i1f[:],
                                         start=True, stop=True)
                        rinv = sc1.tile([QH * Q, 1], F32, name="rinv", tag="rinv")
                        nc.vector.reciprocal(rinv[:], rsT_ps[:])
                        aob = sc1.tile([QH * Q, HD], BF16, name="aob", tag="aob")
                        nc.vector.tensor_scalar_mul(aob[:], ao_ps[:], rinv[:])
                        aoT_ps = ktp2.tile([128, QH * Q], BF16, name="aoT_ps",
                                           tag="kt_ps")
                        nc.tensor.transpose(aoT_ps[:], aob[:], ident[:QH * Q, :QH * Q])
                        nc.vector.tensor_copy(
                            attn_cat.rearrange("p (h bt) -> p h bt", bt=2 * Q)
                                [:, :, Q * b:Q * (b + 1)],
                            aoT_ps.rearrange("p (h t) -> p h t", t=Q)[:])
                    # o projection (ow prefetched before attention)
                    obf = arb.tile([DTOK, H], BF16, name="obf", tag="obf")
                    o_ps = {nn: dps.tile([DTOK, 512], F32, name="o_ps", tag="acc")
                            for nn in range(5)}
                    for h in range(QH):
                        ow_blk = ow_blks[0 if h < 3 else 1]
                        hl = h if h < 3 else h - 3
                        for nn in range(5):
                            nc.tensor.matmul(
                                o_ps[nn][:], attn_cat[:, 32 * h:32 * (h + 1)],
                                ow_blk[:, hl, 512 * nn:512 * (nn + 1)],
                                start=(h == 0), stop=(h == QH - 1))
                    for nn in range(5):
                        nc.vector.tensor_copy(obf[:, 512 * nn:512 * (nn + 1)],
                                              o_ps[nn][:])
                    oin = ardr.tile([DTOK, H], BF16, name="oin", tag="oin")
                    oout = ardr.tile([DTOK, H], BF16, name="oout", tag="oout")
                    nc.sync.dma_start(out=oin[:], in_=obf[:])
                    nc.gpsimd.collective_compute("AllReduce", OP.add,
                                                 replica_groups=GROUPS,
                                                 ins=[oin[:]], outs=[oout[:]])
                    oas = arb.tile([DTOK, H], BF16, name="oas", tag="oas")
                    nc.sync.dma_start(out=oas[:], in_=oout[:])
                    nc.vector.tensor_tensor(out=x_sb[:], in0=x_sb[:], in1=oas[:],
                                            op=OP.add)
                    # inv_rms2 + x2T
                    xsq2 = sc1.tile([DTOK, H], BF16, name="xsq2", tag="xb")
                    ssx2 = sc1.tile([DTOK, 1], F32, name="ssx2", tag="ssx")
                    nc.scalar.activation(xsq2[:], x_sb[:], AF.Square,
                                         accum_out=ssx2[:])
                    inv2 = _rsqrt(nc, sc1, ssx2, H, DTOK, "in2")
                    xb2 = sc1.tile([DTOK, H], BF16, name="xb2", tag="xb")
                    nc.vector.tensor_copy(xb2[:], x_sb[:])
                    x2T = decx.tile([128, NKC, DTOK], BF16, name="x2T", tag="x2T")
                    for k in range(NKC):
                        tp2 = ktp2.tile([128, DTOK], BF16, name="tp_x2", tag="kt_ps")
                        nc.tensor.transpose(tp2[:], xb2[:, 128 * k:128 * (k + 1)],
                                            ident[:DTOK, :DTOK])
                        nc.vector.tensor_copy(x2T[:, k, :], tp2[:])
                    # gate/up — 4 block DMAs of 5 k-chunks, 7 psums live
                    h_sb = sc1.tile([DTOK, ISH], BF16, name="h_sb", tag="h_sb")
                    gu_off = [0, 512, 1024, 1536, 2048, 2560, 3072]
                    gu_ps = {j: dps.tile([DTOK, GU_CH[j]], F32, name=f"gu{j}",
                                         tag="acc") for j in range(7)}
                    for kb in range(7):
                        gln = 2 if kb == 6 else 3
                        gu_w = wblkp.tile([128, 3, GU2], BF16, name="gu_w",
                                          tag="wblk")
                        nc.sync.dma_start(out=gu_w[:, :gln, :],
                                          in_=guw_r[l, :, 3 * kb:3 * kb + gln, :])
                        for k5 in range(gln):
                            k = 3 * kb + k5
                            for j in range(7):
                                nc.tensor.matmul(
                                    gu_ps[j][:], x2T[:, k, :],
                                    gu_w[:, k5, gu_off[j]:gu_off[j] + GU_CH[j]],
                                    start=(k == 0), stop=(k == NKC - 1))
                    # consume psums: pairs (g, u)
                    for j in range(0, 7, 2):
                        if j < 6:
                            w = GU_CH[j]
                            g_ap = gu_ps[j][:]
                            u_ap = gu_ps[j + 1][:]
                        else:  # combined [g3 | u3] chunk
                            w = 192
                            g_ap = gu_ps[6][:, 0:192]
                            u_ap = gu_ps[6][:, 192:384]
                        hcol = gu_off[j] // 2
                        sg = sc1.tile([DTOK, 512], BF16, name="sg", tag="sg")
                        nc.scalar.activation(sg[:, :w], g_ap, AF.Silu,
                                             scale=inv2[:])
                        uh = sc1.tile([DTOK, 512], BF16, name="uh", tag="uh")
                        nc.vector.tensor_scalar_mul(uh[:, :w], u_ap, inv2[:])
                        nc.vector.tensor_tensor(
                            out=h_sb[:, hcol:hcol + w],
                            in0=sg[:, :w], in1=uh[:, :w], op=OP.mult)
                    # hT
                    hT = decx.tile([128, DKC, DTOK], BF16, name="hT", tag="hT")
                    for k in range(DKC):
                        klen = 128 if k < DKC - 1 else DK_LAST
                        hp = ktp2.tile([128, DTOK], BF16, name="hp", tag="kt_ps")
                        nc.tensor.transpose(hp[:klen, :],
                                            h_sb[:, 128 * k:128 * k + klen],
                                            ident[:DTOK, :DTOK])
                        nc.vector.tensor_copy(hT[:klen, k, :], hp[:klen, :])
                    # down — 2 block DMAs of 7 k-chunks (weights zero-padded)
                    dbf = arb.tile([DTOK, H], BF16, name="dbf", tag="obf")
                    d_ps = {nn: dps.tile([DTOK, 512], F32, name="d_ps", tag="acc")
                            for nn in range(5)}
                    db_off = [0, 4, 8, 12]
                    db_len = [4, 4, 4, 2]
                    for db in range(4):
                        dw_b = wblkp.tile([128, 4, H], BF16, name="dw_b",
                                          tag="wblk")
                        o0, ln = db_off[db], db_len[db]
                        nc.sync.dma_start(out=dw_b[:, :ln, :],
                                          in_=dw_r[l, :, o0:o0 + ln, :])
                        for k7 in range(ln):
                            k = o0 + k7
                            klen = 128 if k < DKC - 1 else DK_LAST
                            for nn in range(5):
                                nc.tensor.matmul(
                                    d_ps[nn][:], hT[:klen, k, :],
                                    dw_b[:klen, k7, 512 * nn:512 * (nn + 1)],
                                    start=(k == 0), stop=(k == DKC - 1))
                    for nn in range(5):
                        nc.vector.tensor_copy(dbf[:, 512 * nn:512 * (nn + 1)],
                                              d_ps[nn][:])
                    din = ardr.tile([DTOK, H], BF16, name="din", tag="oin")
                    dout = ardr.tile([DTOK, H], BF16, name="dout", tag="oout")
                    nc.sync.dma_start(out=din[:], in_=dbf[:])
                    nc.gpsimd.collective_compute("AllReduce", OP.add,
                                                 replica_groups=GROUPS,
                                                 ins=[din[:]], outs=[dout[:]])
                    das = arb.tile([DTOK, H], BF16, name="das", tag="oas")
                    nc.sync.dma_start(out=das[:], in_=dout[:])
                    nc.vector.tensor_tensor(out=x_sb[:], in0=x_sb[:], in1=das[:],
                                            op=OP.add)

                # final norm (norm_w applied on host)
                xsqf = sc1.tile([DTOK, H], BF16, name="xsqf", tag="xb")
                ssf = sc1.tile([DTOK, 1], F32, name="ssf", tag="ssx")
                nc.scalar.activation(xsqf[:], x_sb[:], AF.Square,
                                     accum_out=ssf[:])
                invf = _rsqrt(nc, sc1, ssf, H, DTOK, "fin")
                y_sb = arb.tile([DTOK, H], F32, name="y_sb", tag="ysb")
                nc.vector.tensor_scalar_mul(y_sb[:], x_sb[:], invf[:])
                nc.sync.dma_start(out=y_d[:], in_=y_sb[:])



_NC_CACHE = None


def _get_nc():
    global _NC_CACHE
    if _NC_CACHE is None:
        _NC_CACHE = _build()
    return _NC_CACHE


def _prep_inputs(noise_embedding, target_hidden, position_ids, fc_w,
                 hidden_norm_w, q_w, k_w, v_w, o_w, qn_w, kn_w, gate_w, up_w,
                 down_w, ln1_w, ln2_w, norm_w):
    bf = ml_dtypes.bfloat16
    pos = np.asarray(position_ids)
    inv_freq = (1.0 / (THETA ** (np.arange(0, HD, 2, dtype=np.float64) / HD)))

    def cos_sin(p):  # p: (n,) positions -> cos/sin (n, HD) float32
        ang = p.astype(np.float64)[:, None] * inv_freq[None, :]
        c = np.cos(ang); s = np.sin(ang)
        return (np.concatenate([c, c], -1).astype(np.float32),
                np.concatenate([s, s], -1).astype(np.float32))

    qw = np.asarray(q_w); kw = np.asarray(k_w); vw = np.asarray(v_w)
    ow = np.asarray(o_w); gw = np.asarray(gate_w); uw = np.asarray(up_w)
    dw = np.asarray(down_w); fw = np.asarray(fc_w)
    ln1 = np.asarray(ln1_w); ln2 = np.asarray(ln2_w)
    hw = np.asarray(hidden_norm_w)
    qn = np.asarray(qn_w); kn = np.asarray(kn_w)
    th_in = np.asarray(target_hidden)
    ne = np.asarray(noise_embedding)

    in_maps = []
    # per-rank weight tensors (shared across the two DP groups)
    rank_data = []
    for t in range(TP):
        fcw_t = np.ascontiguousarray(
            fw[640 * t:640 * (t + 1), :].T).astype(bf)       # [12800, 640]
        kvw_t = np.empty((L, H, 256), np.float32)
        qkvw_t = np.empty((L, H, 896), np.float32)
        ow_t = np.empty((L, HSH, H), np.float32)
        guw_t = np.empty((L, H, GU2), np.float32)
        dw_t = np.zeros((L, DWP, H), np.float32)
        for l in range(L):
            kslc = kw[l, HD * t:HD * (t + 1), :] * hw[None, :]
            vslc = vw[l, HD * t:HD * (t + 1), :] * hw[None, :]
            kvw_t[l] = np.concatenate([kslc, vslc], 0).T
            qs = qw[l, 640 * t:640 * (t + 1), :] * ln1[l][None, :]
            kds = kw[l, HD * t:HD * (t + 1), :] * ln1[l][None, :]
            vds = vw[l, HD * t:HD * (t + 1), :] * ln1[l][None, :]
            qkvw_t[l] = np.concatenate([qs, kds, vds], 0).T
            ow_t[l] = ow[l][:, 640 * t:640 * (t + 1)].T
            g = gw[l, ISH * t:ISH * (t + 1), :] * ln2[l][None, :]
            u = uw[l, ISH * t:ISH * (t + 1), :] * ln2[l][None, :]
            # interleave gate/up in 512-col pair chunks (g0,u0,g1,u1,g2,u2,g3,u3)
            parts = []
            for j, wdt in enumerate([512, 512, 512, 192]):
                o0 = 512 * j
                parts.append(g[o0:o0 + wdt, :])
                parts.append(u[o0:o0 + wdt, :])
            guw_t[l] = np.concatenate(parts, 0).T
            dw_t[l, :ISH] = dw[l][:, ISH * t:ISH * (t + 1)].T
        rank_data.append(dict(
            fcw=fcw_t, kvw=kvw_t.astype(bf), qkvw=qkvw_t.astype(bf),
            ow=ow_t.astype(bf), guw=guw_t.astype(bf), dw=dw_t.astype(bf)))

    group_data = []
    for g in range(2):
        bsel = [2 * g, 2 * g + 1]
        xt_g = np.ascontiguousarray(
            th_in[bsel].transpose(2, 0, 1).reshape(FIN, TOK)).astype(bf)
        x0_g = np.ascontiguousarray(ne[bsel].reshape(DTOK, H), np.float32)
        ckc = np.empty((L, 2, CTX, HD), np.float32)
        skc = np.empty((L, 2, CTX, HD), np.float32)
        ckq = np.empty((L, DTOK, HD), np.float32)
        skq = np.empty((L, DTOK, HD), np.float32)
        ckd = np.empty((L, DTOK, HD), np.float32)
        skd = np.empty((L, DTOK, HD), np.float32)
        for bi, b in enumerate(bsel):
            cc, ss = cos_sin(pos[b, :CTX])
            cd, sd = cos_sin(pos[b, CTX:S])
            for l in range(L):
                knl = kn[l]; qnl = qn[l]
                # k tables: CK = kn*cos; SK[:64] = -kn[64:]*sin[:64],
                #           SK[64:] = kn[:64]*sin[64:]
                ckc[l, bi] = cc * knl[None, :]
                skc[l, bi, :, :64] = -ss[:, :64] * knl[None, 64:]
                skc[l, bi, :, 64:] = ss[:, 64:] * knl[None, :64]
                ckd[l, Q * bi:Q * (bi + 1)] = cd * knl[None, :]
                skd[l, Q * bi:Q * (bi + 1), :64] = -sd[:, :64] * knl[None, 64:]
                skd[l, Q * bi:Q * (bi + 1), 64:] = sd[:, 64:] * knl[None, :64]
                sc = 1.0 / np.sqrt(HD)
                ckq[l, Q * bi:Q * (bi + 1)] = cd * qnl[None, :] * sc
                skq[l, Q * bi:Q * (bi + 1), :64] = \
                    -sd[:, :64] * qnl[None, 64:] * sc
                skq[l, Q * bi:Q * (bi + 1), 64:] = \
                    sd[:, 64:] * qnl[None, :64] * sc
        group_data.append(dict(
            xt=xt_g, x0=x0_g, ckc=ckc.astype(bf), skc=skc.astype(bf),
            ckq=ckq.astype(bf), skq=skq.astype(bf), ckd=ckd.astype(bf),
            skd=skd.astype(bf)))

    for core in range(NCORES):
        g, t = core // TP, core % TP
        m = {}
        m.update(rank_data[t])
        m.update(group_data[g])
        in_maps.append(m)
    return in_maps


def kernel(**inputs):
    nc = _get_nc()
    in_maps = _prep_inputs(**inputs)
    res = bass_utils.run_bass_kernel_spmd(
        nc, in_maps, core_ids=list(range(NCORES)), trace=False)
    norm_w = np.asarray(inputs["norm_w"]).astype(np.float32)
    y0 = res.results[0]["y"]
    y1 = res.results[TP]["y"]
    y = np.concatenate([y0, y1], 0) * norm_w[None, :]
    return y.reshape(B, Q, H).astype(np.float32)



# revision 10
# speedup vs baseline: 1.4256x; 1.4256x over previous
"""DFlashDraftModel Trainium2 kernel — 8 NeuronCores, DP2 x TP4 (v3).

v3 vs v2 baseline:
- fc matmul in fp8 (e4m3) DoubleRow (256-row contraction/MM, ~2x PE rate);
  stationary fcw chunk streams two 512-token chunks to amortize LDWEIGHTS.
- KV-build matmuls fp8 DoubleRow (th carried as fp8 through the AllGather;
  all fp8 scales cancel through k-head RMS and the deferred inv-rms(th)
  scaling of V, except a 1/SKV factor folded into invT's sqrt scale).
- Decoder MLP tensor-parallel over all 8 cores (gate/up 864 cols/core, down
  input-sharded 864) on all 64 tokens; o/down AllReduce become 8-core ARs of
  [64, H].  Group-dependent row placement is done with per-core selection /
  placement matrices supplied as inputs (qsel/osel), never static slices, so
  one SPMD program serves both DP groups.
- q/k/v + rope computed for all 64 tokens (M dim is free on the PE).

Decoder matmuls stay bf16 (fp8 there fails the accuracy budget).
"""
import sys

if '/opt/trn_rl_repo' not in sys.path:
    sys.path.insert(0, '/opt/trn_rl_repo')

import numpy as np
import ml_dtypes

import concourse.bass as bass
import concourse.tile as tile
from concourse import bacc, mybir
from concourse import bass_utils
from concourse.masks import make_identity

BF16 = mybir.dt.bfloat16
F32 = mybir.dt.float32
F8 = mybir.dt.float8e4
AF = mybir.ActivationFunctionType
OP = mybir.AluOpType
DR = mybir.MatmulPerfMode.DoubleRow

L = 5; B = 4; Q = 16; CTX = 2048; S = CTX + Q
H = 2560; I = 6912; NH = 20; NKV = 4; HD = 128; NF = 5
EPS = 1e-6; THETA = 1000000.0

NCORES = 8
TP = 4
GROUPS = [[0, 1, 2, 3], [4, 5, 6, 7]]
ALL8 = [[0, 1, 2, 3, 4, 5, 6, 7]]
TOK = 2 * CTX               # ctx tokens per DP group
NTC = TOK // 128            # 32 context token chunks
FIN = NF * H                # 12800
NJP = FIN // 256            # 50 fc contraction chunk-pairs
HSH = H // TP               # 640 fc output shard per core
QH = NH // TP               # 5 q heads per core
DT2 = 4 * Q                 # 64 decoder tokens (both groups)
NKC = H // 128              # 20
NKP = NKC // 2              # 10 kv contraction chunk-pairs
NNP = 4                     # fc token chunk-pairs (1024 tokens each)
ISH8 = I // 8               # 864
GU2 = 2 * ISH8              # 1728
DKC = 7                     # down contraction chunks (864 pad 896)
DK_LAST = ISH8 - 6 * 128    # 96
DWP = DKC * 128             # 896

# fp8 scales (host-applied, power-of-two).
SA = 16.0                   # xt scale
SB = 512.0                  # fc_w scale
SKV = 512.0                 # kv ctx weight scale
EVP = 8.0 / (SA * SB * 2.26)   # fc psum -> th8 evict scale
CVQ = 1.0 / (SKV * SKV)     # inside-sqrt scale for invT


def _build():
    nc = bacc.Bacc("TRN2", target_bir_lowering=False, debug=False,
                   enable_asserts=False, num_devices=NCORES)

    xt8_d = nc.dram_tensor("xt8", [NJP, 128, 2, TOK], F8, kind="ExternalInput").ap()
    fcw8_d = nc.dram_tensor("fcw8", [128, NJP, 2, HSH], F8, kind="ExternalInput").ap()
    kvw8_d = nc.dram_tensor("kvw8", [128, NKP, 2, L * 256], F8, kind="ExternalInput").ap()
    qkvw_d = nc.dram_tensor("qkvw", [L, H, 896], BF16, kind="ExternalInput").ap()
    ow_d = nc.dram_tensor("ow", [L, HSH, H], BF16, kind="ExternalInput").ap()
    guw_d = nc.dram_tensor("guw", [L, H, GU2], BF16, kind="ExternalInput").ap()
    dw_d = nc.dram_tensor("dw", [L, DWP, H], BF16, kind="ExternalInput").ap()
    ckc_d = nc.dram_tensor("ckc", [L, 2, CTX, HD], BF16, kind="ExternalInput").ap()
    skc_d = nc.dram_tensor("skc", [L, 2, CTX, HD], BF16, kind="ExternalInput").ap()
    ckq_d = nc.dram_tensor("ckq", [L, DT2, HD], BF16, kind="ExternalInput").ap()
    skq_d = nc.dram_tensor("skq", [L, DT2, HD], BF16, kind="ExternalInput").ap()
    ckd_d = nc.dram_tensor("ckd", [L, DT2, HD], BF16, kind="ExternalInput").ap()
    skd_d = nc.dram_tensor("skd", [L, DT2, HD], BF16, kind="ExternalInput").ap()
    qsel_d = nc.dram_tensor("qsel", [DT2, 2 * Q], BF16, kind="ExternalInput").ap()
    osel_d = nc.dram_tensor("osel", [2, QH * Q, QH * DT2], BF16, kind="ExternalInput").ap()
    x0_d = nc.dram_tensor("x0", [DT2, H], F32, kind="ExternalInput").ap()
    y_d = nc.dram_tensor("y", [DT2, H], F32, kind="ExternalOutput").ap()

    with tile.TileContext(nc) as tc:
        _emit(nc, tc, xt8_d, fcw8_d, kvw8_d, qkvw_d, ow_d, guw_d, dw_d,
              ckc_d, skc_d, ckq_d, skq_d, ckd_d, skd_d, qsel_d, osel_d,
              x0_d, y_d)
    nc.compile()
    return nc


def _rsqrt(nc, pool, ss, n_mean, ptok, name, sqrt_scale=1.0):
    """sqrt(sqrt_scale) * rsqrt(ss/n_mean + EPS), per-partition scalars."""
    m = pool.tile([ptok, 1], F32, name=f"m_{name}", tag=f"rq_{name}m")
    nc.vector.tensor_scalar(out=m[:], in0=ss[:], scalar1=1.0 / n_mean,
                            scalar2=EPS, op0=OP.mult, op1=OP.add)
    r = pool.tile([ptok, 1], F32, name=f"r_{name}", tag=f"rq_{name}r")
    nc.vector.reciprocal(r[:], m[:])
    o = pool.tile([ptok, 1], F32, name=f"o_{name}", tag=f"rq_{name}o")
    if sqrt_scale == 1.0:
        nc.scalar.sqrt(o[:], r[:])
    else:
        nc.scalar.activation(o[:], r[:], AF.Sqrt, scale=sqrt_scale)
    return o


def _rope(nc, pool, src, ck, sk, ptok, name):
    """out = src*ck + rotate_half_pair(src)*sk (sign folded into sk)."""
    t1 = pool.tile([ptok, HD], BF16, name=f"t1_{name}", tag="rope_t1")
    nc.vector.tensor_tensor(out=t1[:], in0=src, in1=ck, op=OP.mult)
    t2 = pool.tile([ptok, HD], BF16, name=f"t2_{name}", tag="rope_t2")
    nc.vector.tensor_tensor(out=t2[:, 0:64], in0=src[:, 64:128],
                            in1=sk[:, 0:64], op=OP.mult)
    nc.vector.tensor_tensor(out=t2[:, 64:128], in0=src[:, 0:64],
                            in1=sk[:, 64:128], op=OP.mult)
    t3 = pool.tile([ptok, HD], BF16, name=f"t3_{name}", tag="rope_t3")
    nc.vector.tensor_tensor(out=t3[:], in0=t1[:], in1=t2[:], op=OP.add)
    return t3


def _emit(nc, tc, xt8_d, fcw8_d, kvw8_d, qkvw_d, ow_d, guw_d, dw_d,
          ckc_d, skc_d, ckq_d, skq_d, ckd_d, skd_d, qsel_d, osel_d,
          x0_d, y_d):
    with tc.tile_pool(name="glob", bufs=1) as glob, \
         tc.tile_pool(name="dramg", bufs=1, space="DRAM") as dram:
        ident = glob.tile([128, 128], BF16, name="ident")
        make_identity(nc, ident[:])
        ones = glob.tile([128, 1], BF16, name="ones")
        nc.vector.memset(ones[:], 1.0)
        i1f = glob.tile([1, 1], F32, name="i1f")
        nc.vector.memset(i1f[:], 1.0)
        x_sb = glob.tile([DT2, H], F32, name="x_sb")
        nc.sync.dma_start(out=x_sb[:], in_=x0_d[:])
        qsel = glob.tile([DT2, 2 * Q], BF16, name="qsel")
        nc.sync.dma_start(out=qsel[:], in_=qsel_d[:])
        osel = glob.tile([QH * Q, 2, QH * DT2], BF16, name="osel")
        nc.sync.dma_start(out=osel[:],
                          in_=osel_d.rearrange("b r c -> r b c"))
        invT = glob.tile([128, NTC], F32, name="invT")

        th8_n = [dram.tile([H, 1024], F8, name=f"th8_{n}") for n in range(NNP)]
        ssq_in = dram.tile([1, TOK], F32, name="ssq_in")
        ssq_out = dram.tile([1, TOK], F32, name="ssq_out")

        # ---------------- Phase 1: fc (fp8 DoubleRow) ----------------
        with tc.tile_pool(name="fcw_pool", bufs=1) as fcwp, \
             tc.tile_pool(name="xtp", bufs=52) as xtp, \
             tc.tile_pool(name="fcps", bufs=6, space="PSUM") as fcps, \
             tc.tile_pool(name="ssqps", bufs=1, space="PSUM") as ssqps, \
             tc.tile_pool(name="fcev", bufs=4) as fcev, \
             tc.tile_pool(name="ssqrp", bufs=1) as ssqrp, \
             tc.tile_pool(name="agp", bufs=2, space="DRAM") as agp:
            fcw8_sb = fcwp.tile([128, NJP, 2, HSH], F8)
            ssq_row = ssqrp.tile([1, TOK], F32, name="ssq_row")
            for q5 in range(5):
                nc.sync.dma_start(
                    out=fcw8_sb[:, 10 * q5:10 * (q5 + 1), :, :],
                    in_=fcw8_d[:, 10 * q5:10 * (q5 + 1), :, :])
            for np_ in range(NNP):
                ag_in = agp.tile([HSH, 1024], F8, name="ag_in", tag="ag_in")
                ssq_ps = [ssqps.tile([1, 512], F32, name=f"ssq_ps{h}",
                                     tag=f"ssq{h}") for h in range(2)]
                xts = []
                for ms in ([0, 1, 2], [3, 4]):
                    psF = {(m, h): fcps.tile([128, 512], F32, name="psF",
                                             tag="psF")
                           for m in ms for h in range(2)}
                    for j in range(NJP):
                        if len(xts) <= j:
                            xt_t = xtp.tile([128, 2, 1024], F8, name="xt_t",
                                            tag="xt_t")
                            nc.sync.dma_start(
                                out=xt_t[:],
                                in_=xt8_d[j, :, :,
                                          1024 * np_:1024 * (np_ + 1)])
                            xts.append(xt_t)
                        xt_t = xts[j]
                        for m in ms:
                            lhs = fcw8_sb[:, j, :, 128 * m:128 * (m + 1)]
                            nc.tensor.matmul(
                                psF[(m, 0)][:], lhs, xt_t[:, :, 0:512],
                                perf_mode=DR,
                                start=(j == 0), stop=(j == NJP - 1))
                            nc.tensor.matmul(
                                psF[(m, 1)][:], lhs, xt_t[:, :, 512:1024],
                                perf_mode=DR,
                                start=(j == 0), stop=(j == NJP - 1))
                    for m in ms:
                        for h in range(2):
                            th8 = fcev.tile([128, 512], F8, name="th8",
                                            tag="th8")
                            nc.vector.tensor_scalar(
                                out=th8[:], in0=psF[(m, h)][:],
                                scalar1=EVP, scalar2=None, op0=OP.mult)
                            nc.sync.dma_start(
                                out=ag_in[128 * m:128 * (m + 1),
                                          512 * h:512 * (h + 1)],
                                in_=th8[:])
                            th2 = fcev.tile([128, 512], BF16, name="th2",
                                            tag="th2")
                            nc.scalar.square(th2[:], th8[:])
                            nc.tensor.matmul(ssq_ps[h][:], ones[:], th2[:],
                                             start=(m == 0), stop=(m == 4))
                for h in range(2):
                    nc.vector.tensor_copy(
                        ssq_row[:, 1024 * np_ + 512 * h:
                                1024 * np_ + 512 * (h + 1)],
                        ssq_ps[h][:])
                nc.gpsimd.collective_compute(
                    "AllGather", OP.bypass, replica_groups=GROUPS,
                    ins=[ag_in[:]], outs=[th8_n[np_][:]])
            nc.sync.dma_start(out=ssq_in[:], in_=ssq_row[:])
            nc.gpsimd.collective_compute(
                "AllReduce", OP.add, replica_groups=GROUPS,
                ins=[ssq_in[:]], outs=[ssq_out[:]])

        with tc.tile_pool(name="kvglob", bufs=1) as kvglob:
            KT_all = kvglob.tile([128, L, 2, S], BF16, name="KT_all")
            V_all = kvglob.tile([128, L, 2, 17, HD], BF16, name="V_all")
            # ---------------- Phase 2: KV build (fp8 DR) ----------------
            with tc.tile_pool(name="kvwp", bufs=1) as kvwp, \
                 tc.tile_pool(name="tht", bufs=2) as thtp, \
                 tc.tile_pool(name="ropep", bufs=2) as ropep, \
                 tc.tile_pool(name="kvps", bufs=2, space="PSUM") as kvps, \
                 tc.tile_pool(name="ktps", bufs=2, space="PSUM") as ktps, \
                 tc.tile_pool(name="kvpost", bufs=3) as kvpost:
                kvw8_sb = kvwp.tile([128, NKP, 2, L * 256], F8, name="kvw8_sb")
                for q2 in range(2):
                    nc.sync.dma_start(
                        out=kvw8_sb[:, 5 * q2:5 * (q2 + 1), :, :],
                        in_=kvw8_d[:, 5 * q2:5 * (q2 + 1), :, :])
                ckc_r = ckc_d.rearrange("l b (c p) d -> p l (b c) d", p=128)
                skc_r = skc_d.rearrange("l b (c p) d -> p l (b c) d", p=128)
                for t_ in range(NTC):
                    if t_ == 10:
                        with tc.tile_pool(name="ivt2", bufs=1) as ivt2:
                            ssT = ivt2.tile([128, NTC], F32, name="ssT")
                            nc.sync.dma_start(
                                out=ssT[:],
                                in_=ssq_out.rearrange("o (c p) -> (o p) c",
                                                      p=128))
                            m = ivt2.tile([128, NTC], F32, name="m_iv")
                            nc.vector.tensor_scalar(out=m[:], in0=ssT[:],
                                                    scalar1=1.0 / H,
                                                    scalar2=EPS,
                                                    op0=OP.mult, op1=OP.add)
                            r = ivt2.tile([128, NTC], F32, name="r_iv")
                            nc.vector.reciprocal(r[:], m[:])
                            nc.scalar.activation(invT[:], r[:], AF.Sqrt,
                                                 scale=CVQ)
                    b, c = t_ // 16, t_ % 16
                    thT8 = thtp.tile([128, NKP, 2, 128], F8, name="thT8",
                                     tag="thT")
                    nc.sync.dma_start(
                        out=thT8[:],
                        in_=th8_n[t_ // 8].rearrange(
                            "(j i p) (cc w) -> p j i cc w", i=2, p=128, w=128)
                            [:, :, :, t_ % 8, :])
                    ck_t = ropep.tile([128, L, HD], BF16, name="ck_t",
                                      tag="ck_t")
                    nc.sync.dma_start(out=ck_t[:], in_=ckc_r[:, :, t_, :])
                    sk_t = ropep.tile([128, L, HD], BF16, name="sk_t",
                                      tag="sk_t")
                    nc.sync.dma_start(out=sk_t[:], in_=skc_r[:, :, t_, :])
                    P01 = kvps.tile([128, 512], F32, name="P01", tag="P01")
                    P23 = kvps.tile([128, 512], F32, name="P23", tag="P23")
                    P4 = kvps.tile([128, 256], F32, name="P4", tag="P4")
                    for j in range(NKP):
                        lhs = thT8[:, j, :, :]
                        nc.tensor.matmul(P01[:], lhs,
                                         kvw8_sb[:, j, :, 0:512],
                                         perf_mode=DR,
                                         start=(j == 0), stop=(j == NKP - 1))
                        nc.tensor.matmul(P23[:], lhs,
                                         kvw8_sb[:, j, :, 512:1024],
                                         perf_mode=DR,
                                         start=(j == 0), stop=(j == NKP - 1))
                        nc.tensor.matmul(P4[:], lhs,
                                         kvw8_sb[:, j, :, 1024:1280],
                                         perf_mode=DR,
                                         start=(j == 0), stop=(j == NKP - 1))
                    srcs = [(P01, 0), (P01, 256), (P23, 0), (P23, 256),
                            (P4, 0)]
                    khn_all = kvpost.tile([128, L * HD], BF16, name="khn_all",
                                          tag="khn")
                    for l in range(L):
                        src, off = srcs[l]
                        sq = kvpost.tile([128, HD], BF16, name="sq", tag="sq")
                        ks = kvpost.tile([128, 1], F32, name="ks", tag="ks")
                        nc.scalar.activation(sq[:], src[:, off:off + 128],
                                             AF.Square, accum_out=ks[:])
                        rs = _rsqrt(nc, kvpost, ks, HD, 128, f"kh{l}")
                        nc.vector.tensor_scalar_mul(
                            khn_all[:, 128 * l:128 * (l + 1)],
                            src[:, off:off + 128], rs[:])
                        nc.vector.tensor_copy(V_all[:, l, b, c, :],
                                              src[:, off + 128:off + 256])
                    kr = kvpost.tile([128, L * HD], BF16, name="kr", tag="kr")
                    nc.vector.tensor_tensor(out=kr[:], in0=khn_all[:],
                                            in1=ck_t[:].rearrange(
                                                "p l d -> p (l d)"),
                                            op=OP.mult)
                    t2 = kvpost.tile([128, L * HD], BF16, name="t2r",
                                     tag="t2r")
                    kh4 = khn_all[:].rearrange("p (l i w) -> p l i w",
                                               i=2, w=64)
                    sk4 = sk_t[:].rearrange("p l (i w) -> p l i w", i=2)
                    t24 = t2[:].rearrange("p (l i w) -> p l i w", i=2, w=64)
                    nc.vector.tensor_tensor(out=t24[:, :, 0, :],
                                            in0=kh4[:, :, 1, :],
                                            in1=sk4[:, :, 0, :], op=OP.mult)
                    nc.vector.tensor_tensor(out=t24[:, :, 1, :],
                                            in0=kh4[:, :, 0, :],
                                            in1=sk4[:, :, 1, :], op=OP.mult)
                    nc.vector.tensor_tensor(out=kr[:], in0=kr[:], in1=t2[:],
                                            op=OP.add)
                    for l in range(L):
                        kt_ps = ktps.tile([128, 128], BF16, name="kt_ps",
                                          tag="kt_ps")
                        nc.tensor.transpose(kt_ps[:],
                                            kr[:, 128 * l:128 * (l + 1)],
                                            ident[:])
                        nc.vector.tensor_copy(
                            KT_all[:, l, b, 128 * c:128 * (c + 1)], kt_ps[:])
                for t_ in range(NTC):
                    b, c = t_ // 16, t_ % 16
                    nc.vector.tensor_scalar_mul(
                        V_all[:, :, b, c, :], V_all[:, :, b, c, :],
                        invT[:, t_:t_ + 1])

            # ---------------- Phase 3: decoder layers ----------------
            with tc.tile_pool(name="sc1", bufs=1) as sc1, \
                 tc.tile_pool(name="arb", bufs=1) as arb, \
                 tc.tile_pool(name="wblk", bufs=3) as wblkp, \
                 tc.tile_pool(name="dps", bufs=6, space="PSUM") as dps, \
                 tc.tile_pool(name="acatp", bufs=1, space="PSUM") as acatp, \
                 tc.tile_pool(name="ktp2", bufs=1, space="PSUM") as ktp2, \
                 tc.tile_pool(name="attp", bufs=2) as attp, \
                 tc.tile_pool(name="decx", bufs=2) as decx, \
                 tc.tile_pool(name="ardr", bufs=2, space="DRAM") as ardr:
                qkvw_r = qkvw_d.rearrange("l (kk p) c -> l p kk c", p=128)
                ow_r = ow_d.rearrange("l (h p) m -> l p h m", p=128)
                guw_r = guw_d.rearrange("l (kk p) m -> l p kk m", p=128)
                dw_r = dw_d.rearrange("l (kk p) m -> l p kk m", p=128)
                for l in range(L):
                    # inv_rms1 of x (all 64 tokens)
                    xsq = sc1.tile([DT2, H], BF16, name="xsq", tag="xb")
                    ssx = sc1.tile([DT2, 1], F32, name="ssx", tag="ssx")
                    nc.scalar.activation(xsq[:], x_sb[:], AF.Square,
                                         accum_out=ssx[:])
                    inv1 = _rsqrt(nc, sc1, ssx, H, DT2, "in1")
                    xb = sc1.tile([DT2, H], BF16, name="xb", tag="xb")
                    nc.vector.tensor_copy(xb[:], x_sb[:])
                    xT = decx.tile([128, NKC, DT2], BF16, name="xT", tag="xT")
                    for k in range(NKC):
                        tp = ktp2.tile([128, DT2], BF16, name="tp_x",
                                       tag="kt_ps")
                        nc.tensor.transpose(tp[:],
                                            xb[:, 128 * k:128 * (k + 1)],
                                            ident[:DT2, :DT2])
                        nc.vector.tensor_copy(xT[:, k, :], tp[:])
                    # qkv projection, all 64 tokens
                    q_ps = dps.tile([DT2, 512], F32, name="q_ps", tag="acc")
                    q_ps2 = dps.tile([DT2, 384], F32, name="q_ps2", tag="acc")
                    for hb in range(2):
                        qkv_w = wblkp.tile([128, 10, 896], BF16, name="qkv_w",
                                           tag="wblk")
                        nc.sync.dma_start(
                            out=qkv_w[:],
                            in_=qkvw_r[l, :, 10 * hb:10 * (hb + 1), :])
                        for k10 in range(10):
                            k = 10 * hb + k10
                            nc.tensor.matmul(q_ps[:], xT[:, k, :],
                                             qkv_w[:, k10, 0:512],
                                             start=(k == 0),
                                             stop=(k == NKC - 1))
                            nc.tensor.matmul(q_ps2[:], xT[:, k, :],
                                             qkv_w[:, k10, 512:896],
                                             start=(k == 0),
                                             stop=(k == NKC - 1))
                    # q: scale by inv1, per-head rms, rope (64 tokens)
                    q1 = sc1.tile([DT2, QH * HD], BF16, name="q1", tag="q1")
                    nc.vector.tensor_scalar_mul(q1[:, 0:512], q_ps[:],
                                                inv1[:])
                    nc.vector.tensor_scalar_mul(q1[:, 512:640],
                                                q_ps2[:, 0:128], inv1[:])
                    ckq_sb = sc1.tile([DT2, HD], BF16, name="ckq_sb",
                                      tag="ckq")
                    skq_sb = sc1.tile([DT2, HD], BF16, name="skq_sb",
                                      tag="skq")
                    nc.sync.dma_start(out=ckq_sb[:], in_=ckq_d[l])
                    nc.sync.dma_start(out=skq_sb[:], in_=skq_d[l])
                    qr = sc1.tile([DT2, QH * HD], BF16, name="qr", tag="qr")
                    for h in range(QH):
                        hs = slice(128 * h, 128 * (h + 1))
                        q2h = sc1.tile([DT2, HD], BF16, name="q2h", tag="q2h")
                        qsh = sc1.tile([DT2, 1], F32, name="qsh", tag="qsh")
                        nc.scalar.activation(q2h[:], q1[:, hs], AF.Square,
                                             accum_out=qsh[:])
                        rqh = _rsqrt(nc, sc1, qsh, HD, DT2, f"qh{h}")
                        qhn = sc1.tile([DT2, HD], BF16, name="qhn", tag="qhn")
                        nc.vector.tensor_scalar_mul(qhn[:], q1[:, hs], rqh[:])
                        qrh = _rope(nc, sc1, qhn[:], ckq_sb[:], skq_sb[:],
                                    DT2, "q")
                        nc.vector.tensor_copy(qr[:, hs], qrh[:])
                    # qT per local batch via qsel (per-core selection input)
                    qT = [sc1.tile([128, QH * Q], BF16, name=f"qT{b}",
                                   tag=f"qT{b}") for b in range(2)]
                    for b in range(2):
                        sel = qsel[:, Q * b:Q * (b + 1)]
                        for h in range(QH):
                            tqp = ktp2.tile([128, Q], F32, name="tqp",
                                            tag="kt_ps")
                            nc.tensor.matmul(tqp[:],
                                             qr[:, 128 * h:128 * (h + 1)],
                                             sel, start=True, stop=True)
                            nc.vector.tensor_copy(qT[b][:, Q * h:Q * (h + 1)],
                                                  tqp[:])
                    # kd/vd (all 64 rows; selection later)
                    kvd = sc1.tile([DT2, 256], BF16, name="kvd", tag="kvd")
                    nc.vector.tensor_scalar_mul(kvd[:], q_ps2[:, 128:384],
                                                inv1[:])
                    k2d = sc1.tile([DT2, HD], BF16, name="k2d", tag="k2d")
                    ksd = sc1.tile([DT2, 1], F32, name="ksd", tag="ksd")
                    nc.scalar.activation(k2d[:], kvd[:, 0:128], AF.Square,
                                         accum_out=ksd[:])
                    rsd = _rsqrt(nc, sc1, ksd, HD, DT2, "kd")
                    khd = sc1.tile([DT2, HD], BF16, name="khd", tag="khd")
                    nc.vector.tensor_scalar_mul(khd[:], kvd[:, 0:128], rsd[:])
                    ckd_sb = sc1.tile([DT2, HD], BF16, name="ckd_sb",
                                      tag="ckd")
                    skd_sb = sc1.tile([DT2, HD], BF16, name="skd_sb",
                                      tag="skd")
                    nc.sync.dma_start(out=ckd_sb[:], in_=ckd_d[l])
                    nc.sync.dma_start(out=skd_sb[:], in_=skd_d[l])
                    krd = _rope(nc, sc1, khd[:], ckd_sb[:], skd_sb[:],
                                DT2, "kd")
                    for b in range(2):
                        sel = qsel[:, Q * b:Q * (b + 1)]
                        kdp = ktp2.tile([128, Q], F32, name="kdp",
                                        tag="kt_ps")
                        nc.tensor.matmul(kdp[:], krd[:], sel,
                                         start=True, stop=True)
                        nc.vector.tensor_copy(KT_all[:, l, b, CTX:S], kdp[:])
                        vdp = ktp2.tile([Q, HD], F32, name="vdp",
                                        tag="kt_ps")
                        nc.tensor.matmul(vdp[:], sel, kvd[:, 128:256],
                                         start=True, stop=True)
                        nc.vector.tensor_copy(V_all[0:Q, l, b, 16, :],
                                              vdp[:])
                    # prefetch o weights before attention
                    ow_blks = []
                    for ob in range(2):
                        o0, oln = 3 * ob, (3 if ob == 0 else 2)
                        owt = wblkp.tile([128, 3, H], BF16, name="ow_sb",
                                         tag="wblk")
                        nc.sync.dma_start(out=owt[:, :oln, :],
                                          in_=ow_r[l, :, o0:o0 + oln, :])
                        ow_blks.append(owt)
                    # attention per local batch; placement into [64]-wide
                    # attn_cat via osel input
                    acat_ps = acatp.tile([128, QH * DT2], F32,
                                         name="acat_ps", tag="acat")
                    for b in range(2):
                        rs_ps = dps.tile([1, QH * Q], F32, name="rs_ps",
                                         tag="acc")
                        ao_ps = dps.tile([QH * Q, HD], F32, name="ao_ps",
                                         tag="acc")
                        for s_ in range(17):
                            klen = 128 if s_ < 16 else Q
                            sc_ps = dps.tile([128, QH * Q], F32, name="sc_ps",
                                             tag="acc")
                            nc.tensor.matmul(
                                sc_ps[:klen, :],
                                KT_all[:, l, b, 128 * s_:128 * s_ + klen],
                                qT[b][:], start=True, stop=True)
                            at = attp.tile([128, QH * Q], BF16, name="at",
                                           tag="at")
                            nc.scalar.activation(at[:klen, :], sc_ps[:klen, :],
                                                 AF.Exp)
                            nc.tensor.matmul(rs_ps[:], ones[:klen, :],
                                             at[:klen, :],
                                             start=(s_ == 0), stop=(s_ == 16))
                            nc.tensor.matmul(ao_ps[:], at[:klen, :],
                                             V_all[:klen, l, b, s_, :],
                                             start=(s_ == 0), stop=(s_ == 16))
                        rs_sb = sc1.tile([1, QH * Q], F32, name="rs_sb",
                                         tag="rs_sb")
                        nc.vector.tensor_copy(rs_sb[:], rs_ps[:])
                        rsT_ps = ktp2.tile([QH * Q, 1], F32, name="rsT_ps",
                                           tag="kt_ps")
                        nc.tensor.matmul(rsT_ps[:], rs_sb[:], i1f[:],
                                         start=True, stop=True)
                        rinv = sc1.tile([QH * Q, 1], F32, name="rinv",
                                        tag="rinv")
                        nc.vector.reciprocal(rinv[:], rsT_ps[:])
                        aob = sc1.tile([QH * Q, HD], BF16, name="aob",
                                       tag="aob")
                        nc.vector.tensor_scalar_mul(aob[:], ao_ps[:], rinv[:])
                        nc.tensor.matmul(acat_ps[:], aob[:], osel[:, b, :],
                                         start=(b == 0), stop=(b == 1))
                    acat = sc1.tile([128, QH * DT2], BF16, name="acat",
                                    tag="acat")
                    nc.vector.tensor_copy(acat[:], acat_ps[:])
                    # o projection into [64, H] (other group's rows are zero)
                    obf = arb.tile([DT2, H], BF16, name="obf", tag="obf")
                    o_ps = {nn: dps.tile([DT2, 512], F32, name="o_ps",
                                         tag="acc") for nn in range(5)}
                    for h in range(QH):
                        ow_blk = ow_blks[0 if h < 3 else 1]
                        hl = h if h < 3 else h - 3
                        for nn in range(5):
                            nc.tensor.matmul(
                                o_ps[nn][:], acat[:, DT2 * h:DT2 * (h + 1)],
                                ow_blk[:, hl, 512 * nn:512 * (nn + 1)],
                                start=(h == 0), stop=(h == QH - 1))
                    for nn in range(5):
                        nc.vector.tensor_copy(obf[:, 512 * nn:512 * (nn + 1)],
                                              o_ps[nn][:])
                    oin = ardr.tile([DT2, H], BF16, name="oin", tag="oin")
                    oout = ardr.tile([DT2, H], BF16, name="oout", tag="oout")
                    nc.sync.dma_start(out=oin[:], in_=obf[:])
                    nc.gpsimd.collective_compute("AllReduce", OP.add,
                                                 replica_groups=ALL8,
                                                 ins=[oin[:]], outs=[oout[:]])
                    oas = arb.tile([DT2, H], BF16, name="oas", tag="oas")
                    nc.sync.dma_start(out=oas[:], in_=oout[:])
                    nc.vector.tensor_tensor(out=x_sb[:], in0=x_sb[:],
                                            in1=oas[:], op=OP.add)
                    # inv_rms2 + x2T
                    xsq2 = sc1.tile([DT2, H], BF16, name="xsq2", tag="xb")
                    ssx2 = sc1.tile([DT2, 1], F32, name="ssx2", tag="ssx")
                    nc.scalar.activation(xsq2[:], x_sb[:], AF.Square,
                                         accum_out=ssx2[:])
                    inv2 = _rsqrt(nc, sc1, ssx2, H, DT2, "in2")
                    xb2 = sc1.tile([DT2, H], BF16, name="xb2", tag="xb")
                    nc.vector.tensor_copy(xb2[:], x_sb[:])
                    x2T = decx.tile([128, NKC, DT2], BF16, name="x2T",
                                    tag="x2T")
                    for k in range(NKC):
                        tp2 = ktp2.tile([128, DT2], BF16, name="tp_x2",
                                        tag="kt_ps")
                        nc.tensor.transpose(tp2[:],
                                            xb2[:, 128 * k:128 * (k + 1)],
                                            ident[:DT2, :DT2])
                        nc.vector.tensor_copy(x2T[:, k, :], tp2[:])
                    # gate/up (TP8: 864 each, interleaved [512g,512u,352g,352u])
                    h_sb = sc1.tile([DT2, ISH8], BF16, name="h_sb",
                                    tag="h_sb")
                    gu_ch = [512, 512, 352, 352]
                    gu_off = [0, 512, 1024, 1376]
                    gu_ps = {jj: dps.tile([DT2, gu_ch[jj]], F32,
                                          name=f"gu{jj}", tag="acc")
                             for jj in range(4)}
                    for kb in range(4):
                        gln = 5
                        gu_w = wblkp.tile([128, 5, GU2], BF16, name="gu_w",
                                          tag="wblk")
                        nc.sync.dma_start(
                            out=gu_w[:],
                            in_=guw_r[l, :, 5 * kb:5 * kb + gln, :])
                        for k5 in range(gln):
                            k = 5 * kb + k5
                            for jj in range(4):
                                nc.tensor.matmul(
                                    gu_ps[jj][:], x2T[:, k, :],
                                    gu_w[:, k5,
                                         gu_off[jj]:gu_off[jj] + gu_ch[jj]],
                                    start=(k == 0), stop=(k == NKC - 1))
                    for jj in (0, 2):
                        w = gu_ch[jj]
                        hcol = 0 if jj == 0 else 512
                        sg = sc1.tile([DT2, 512], BF16, name="sg", tag="sg")
                        nc.scalar.activation(sg[:, :w], gu_ps[jj][:], AF.Silu,
                                             scale=inv2[:])
                        uh = sc1.tile([DT2, 512], BF16, name="uh", tag="uh")
                        nc.vector.tensor_scalar_mul(uh[:, :w],
                                                    gu_ps[jj + 1][:], inv2[:])
                        nc.vector.tensor_tensor(
                            out=h_sb[:, hcol:hcol + w],
                            in0=sg[:, :w], in1=uh[:, :w], op=OP.mult)
                    # hT
                    hT = decx.tile([128, DKC, DT2], BF16, name="hT", tag="hT")
                    for k in range(DKC):
                        klen = 128 if k < DKC - 1 else DK_LAST
                        hp = ktp2.tile([128, DT2], BF16, name="hp",
                                       tag="kt_ps")
                        nc.tensor.transpose(hp[:klen, :],
                                            h_sb[:, 128 * k:128 * k + klen],
                                            ident[:DT2, :DT2])
                        nc.vector.tensor_copy(hT[:klen, k, :], hp[:klen, :])
                    # down (864 rows padded to 896; weights zero-padded)
                    dbf = arb.tile([DT2, H], BF16, name="dbf", tag="obf")
                    d_ps = {nn: dps.tile([DT2, 512], F32, name="d_ps",
                                         tag="acc") for nn in range(5)}
                    db_off = [0, 3, 6]
                    db_len = [3, 3, 1]
                    for db in range(3):
                        dw_b = wblkp.tile([128, 3, H], BF16, name="dw_b",
                                          tag="wblk")
                        o0, ln = db_off[db], db_len[db]
                        nc.sync.dma_start(out=dw_b[:, :ln, :],
                                          in_=dw_r[l, :, o0:o0 + ln, :])
                        for k7 in range(ln):
                            k = o0 + k7
                            klen = 128 if k < DKC - 1 else DK_LAST
                            for nn in range(5):
                                nc.tensor.matmul(
                                    d_ps[nn][:], hT[:klen, k, :],
                                    dw_b[:klen, k7, 512 * nn:512 * (nn + 1)],
                                    start=(k == 0), stop=(k == DKC - 1))
                    for nn in range(5):
                        nc.vector.tensor_copy(dbf[:, 512 * nn:512 * (nn + 1)],
                                              d_ps[nn][:])
                    din = ardr.tile([DT2, H], BF16, name="din", tag="oin")
                    dout = ardr.tile([DT2, H], BF16, name="dout", tag="oout")
                    nc.sync.dma_start(out=din[:], in_=dbf[:])
                    nc.gpsimd.collective_compute("AllReduce", OP.add,
                                                 replica_groups=ALL8,
                                                 ins=[din[:]], outs=[dout[:]])
                    das = arb.tile([DT2, H], BF16, name="das", tag="oas")
                    nc.sync.dma_start(out=das[:], in_=dout[:])
                    nc.vector.tensor_tensor(out=x_sb[:], in0=x_sb[:],
                                            in1=das[:], op=OP.add)

                # final norm (norm_w applied on host)
                xsqf = sc1.tile([DT2, H], BF16, name="xsqf", tag="xb")
                ssf = sc1.tile([DT2, 1], F32, name="ssf", tag="ssx")
                nc.scalar.activation(xsqf[:], x_sb[:], AF.Square,
                                     accum_out=ssf[:])
                invf = _rsqrt(nc, sc1, ssf, H, DT2, "fin")
                y_sb = arb.tile([DT2, H], F32, name="y_sb", tag="ysb")
                nc.vector.tensor_scalar_mul(y_sb[:], x_sb[:], invf[:])
                nc.sync.dma_start(out=y_d[:], in_=y_sb[:])


_NC_CACHE = None


def _get_nc():
    global _NC_CACHE
    if _NC_CACHE is None:
        _NC_CACHE = _build()
    return _NC_CACHE


def _f8(x):
    f8 = ml_dtypes.float8_e4m3
    return np.clip(x, -240.0, 240.0).astype(f8)


def _prep_inputs(noise_embedding, target_hidden, position_ids, fc_w,
                 hidden_norm_w, q_w, k_w, v_w, o_w, qn_w, kn_w, gate_w, up_w,
                 down_w, ln1_w, ln2_w, norm_w):
    bf = ml_dtypes.bfloat16
    pos = np.asarray(position_ids)
    inv_freq = (1.0 / (THETA ** (np.arange(0, HD, 2, dtype=np.float64) / HD)))

    def cos_sin(p):  # p: (n,) positions -> cos/sin (n, HD) float32
        ang = p.astype(np.float64)[:, None] * inv_freq[None, :]
        c = np.cos(ang); s = np.sin(ang)
        return (np.concatenate([c, c], -1).astype(np.float32),
                np.concatenate([s, s], -1).astype(np.float32))

    qw = np.asarray(q_w); kw = np.asarray(k_w); vw = np.asarray(v_w)
    ow = np.asarray(o_w); gw = np.asarray(gate_w); uw = np.asarray(up_w)
    dwf = np.asarray(down_w); fw = np.asarray(fc_w)
    ln1 = np.asarray(ln1_w); ln2 = np.asarray(ln2_w)
    hw = np.asarray(hidden_norm_w)
    qn = np.asarray(qn_w); kn = np.asarray(kn_w)
    th_in = np.asarray(target_hidden)
    ne = np.asarray(noise_embedding)

    # ---- per-TP-rank (t = core % 4) tensors ----
    rank_data = []
    for t in range(TP):
        fcw_full = (fw[HSH * t:HSH * (t + 1), :].T * SB)   # [12800, 640]
        fcw8 = _f8(fcw_full).reshape(NJP, 2, 128, HSH).transpose(2, 0, 1, 3)
        kv_cols = []
        qkvw_t = np.empty((L, H, 896), np.float32)
        ow_t = np.empty((L, HSH, H), np.float32)
        for l in range(L):
            kslc = kw[l, HD * t:HD * (t + 1), :] * hw[None, :]
            vslc = vw[l, HD * t:HD * (t + 1), :] * hw[None, :]
            kv_cols.append(np.concatenate([kslc, vslc], 0).T)  # [2560, 256]
            qs = qw[l, 640 * t:640 * (t + 1), :] * ln1[l][None, :]
            kds = kw[l, HD * t:HD * (t + 1), :] * ln1[l][None, :]
            vds = vw[l, HD * t:HD * (t + 1), :] * ln1[l][None, :]
            qkvw_t[l] = np.concatenate([qs, kds, vds], 0).T
            ow_t[l] = ow[l][:, 640 * t:640 * (t + 1)].T
        kv_all = np.concatenate(kv_cols, 1) * SKV           # [2560, 1280]
        kvw8 = _f8(kv_all).reshape(NKP, 2, 128, L * 256).transpose(2, 0, 1, 3)
        rank_data.append(dict(
            fcw8=np.ascontiguousarray(fcw8),
            kvw8=np.ascontiguousarray(kvw8),
            qkvw=qkvw_t.astype(bf), ow=ow_t.astype(bf)))

    # ---- per-core MLP shards (c = core index) ----
    core_mlp = []
    for c in range(NCORES):
        guw_t = np.empty((L, H, GU2), np.float32)
        dw_t = np.zeros((L, DWP, H), np.float32)
        for l in range(L):
            g = gw[l, ISH8 * c:ISH8 * (c + 1), :] * ln2[l][None, :]
            u = uw[l, ISH8 * c:ISH8 * (c + 1), :] * ln2[l][None, :]
            parts = [g[0:512], u[0:512], g[512:864], u[512:864]]
            guw_t[l] = np.concatenate(parts, 0).T
            dw_t[l, :ISH8] = dwf[l][:, ISH8 * c:ISH8 * (c + 1)].T
        core_mlp.append(dict(guw=guw_t.astype(bf), dw=dw_t.astype(bf)))

    # ---- per-DP-group tensors ----
    sc = 1.0 / np.sqrt(HD)
    group_data = []
    for g in range(2):
        bsel = [2 * g, 2 * g + 1]
        xt_g = np.ascontiguousarray(
            th_in[bsel].transpose(2, 0, 1).reshape(FIN, TOK)) * SA
        xt8 = _f8(xt_g).reshape(NJP, 2, 128, TOK).transpose(0, 2, 1, 3)
        ckc = np.empty((L, 2, CTX, HD), np.float32)
        skc = np.empty((L, 2, CTX, HD), np.float32)
        for bi, b in enumerate(bsel):
            cc, ss = cos_sin(pos[b, :CTX])
            for l in range(L):
                knl = kn[l]
                ckc[l, bi] = cc * knl[None, :]
                skc[l, bi, :, :64] = -ss[:, :64] * knl[None, 64:]
                skc[l, bi, :, 64:] = ss[:, 64:] * knl[None, :64]
        # selection inputs
        qsel = np.zeros((DT2, 2 * Q), np.float32)
        for b in range(2):
            for i in range(Q):
                qsel[32 * g + Q * b + i, Q * b + i] = 1.0
        osel = np.zeros((2, QH * Q, QH * DT2), np.float32)
        for b in range(2):
            for h in range(QH):
                for i in range(Q):
                    osel[b, Q * h + i, DT2 * h + Q * (2 * g + b) + i] = 1.0
        group_data.append(dict(
            xt8=np.ascontiguousarray(xt8),
            ckc=ckc.astype(bf), skc=skc.astype(bf),
            qsel=qsel.astype(bf), osel=osel.astype(bf)))

    # ---- rope tables for decoder tokens: all 4 batches [L, 64, HD] ----
    ckq = np.empty((L, DT2, HD), np.float32)
    skq = np.empty((L, DT2, HD), np.float32)
    ckd = np.empty((L, DT2, HD), np.float32)
    skd = np.empty((L, DT2, HD), np.float32)
    for b in range(B):
        cd, sd = cos_sin(pos[b, CTX:S])
        r = slice(Q * b, Q * (b + 1))
        for l in range(L):
            knl = kn[l]; qnl = qn[l]
            ckd[l, r] = cd * knl[None, :]
            skd[l, r, :64] = -sd[:, :64] * knl[None, 64:]
            skd[l, r, 64:] = sd[:, 64:] * knl[None, :64]
            ckq[l, r] = cd * qnl[None, :] * sc
            skq[l, r, :64] = -sd[:, :64] * qnl[None, 64:] * sc
            skq[l, r, 64:] = sd[:, 64:] * qnl[None, :64] * sc
    dec_rope = dict(ckq=ckq.astype(bf), skq=skq.astype(bf),
                    ckd=ckd.astype(bf), skd=skd.astype(bf))
    x0 = np.ascontiguousarray(ne.reshape(DT2, H), np.float32)

    in_maps = []
    for c in range(NCORES):
        g, t = c // TP, c % TP
        m = {}
        m.update(rank_data[t])
        m.update(core_mlp[c])
        m.update(group_data[g])
        m.update(dec_rope)
        m['x0'] = x0
        in_maps.append(m)
    return in_maps


def kernel(**inputs):
    nc = _get_nc()
    in_maps = _prep_inputs(**inputs)
    res = bass_utils.run_bass_kernel_spmd(
        nc, in_maps, core_ids=list(range(NCORES)), trace=False)
    norm_w = np.asarray(inputs["norm_w"]).astype(np.float32)
    y = res.results[0]["y"] * norm_w[None, :]
    return y.reshape(B, Q, H).astype(np.float32)
